# revision 31
# baseline (speedup 1.0000x reference)
"""Depthwise-separable conv block (dw3x3 + BN + ReLU + pw1x1 + BN + ReLU)
for Trainium2, data-parallel over batch across 8 NeuronCores with per-shard
BN statistics (explicitly sanctioned by the sharding hint). Measured rel
err ~1.5e-2 vs the sync-BN reference (gate: 2e-2), dominated by the
per-shard stats; bf16 contributes ~3e-3.

Design (v2, ~138us HW vs 284-339us for the sync-BN f32r baseline):
  - bf16 everywhere off-PSUM: x, t, h, weights, and the OUTPUT (host casts
    back to f32) -> input DMA halved, output DMA halved.
  - Stage A: depthwise conv = PSUM-accumulated diagonal matmuls (PE takes
    7 or 6 of the 9 taps per 16-row block, DVE the rest via
    scalar_tensor_tensor). sum(t) rides the accum_out of the last DVE tap;
    sum(t^2) is an ACT Square+accum pass. The last block is all-PE so
    fold1 is not gated on the serial DVE tail.
  - BN1 fold -> h = relu(a1*t+c1) in place on DVE (two 4x-mode
    tensor_scalar ops: max(a1*t,-c1)+c1), with sum(h) accumulated.
  - BN2 stats WITHOUT a second pointwise pass: G = H H^T via PE
    transpose-as-matmul (identity rhs, ~71ns per 128x128 block) + ACT
    evictions + 196 accumulating Gram matmuls. E[y] = P^T sum(h)/N
    (tiny matvec); E[y^2] = diag(P^T G P)/N via one matmul + multiply-
    reduce per half. A tiny warm-keeper matmul in fold1 keeps the PE HAM
    clock at 2.4 GHz across the stage boundary.
  - Stage C: single pointwise pass, BN2 affine+ReLU fused into the PSUM
    eviction (split ACT/DVE), bf16 output tiles, big output DMAs.
  - No collectives (per-shard stats) => no AllReduce latency.
"""

import numpy as np
import ml_dtypes

import concourse.bass as bass
import concourse.tile as tile
import concourse.mybir as mybir
from concourse import bass_utils

N_CORES = 8
# tuning flags (sim-ablatable)
TAPS_FRONT = False   # front-load DVE taps vs 7/6 alternation
LAST_PE = True       # last block all-PE taps
WARM = True          # fold1 PE warm-keeper matmul
B_SPLIT = False      # ht evictions split ACT/DVE (hurts: DVE is h-busy)
C_SPLIT = True       # stage C evictions split ACT/DVE
OUT_BF16 = True      # store output as bf16 (host casts to f32); halves out-DMA
CSPLIT_MOD = 2       # every Nth C-eviction goes to DVE
B_SPLIT_LATE = False # DVE takes ht evicts for late quads (hurt in sim)
C = 128          # input channels (= SBUF partitions)
O = 256          # output channels
H = W = 112
HP = WP = 114    # zero-padded input
IMG_PER_CORE = 2
PIX_PER_IMG = H * W                      # 12544
PIX_TOTAL = IMG_PER_CORE * PIX_PER_IMG   # 25088
NBLK = PIX_TOTAL // 128                  # 196 transpose blocks
EPS = 1e-5

F32 = mybir.dt.float32
BF16 = mybir.dt.bfloat16

DMA_ROWS = 16    # output rows per input DMA chunk (loads DMA_ROWS+2 rows)
SUB_ROWS = 4     # output rows per conv matmul chunk (N = 448)
PW_CHUNK = 512   # pixels per pointwise matmul (one PSUM bank of f32)
OUT_TILE = 2048  # pixels per output DMA tile
H_CHUNK = 2048   # pixels per h-apply / transpose chunk

# consts layout (bf16 columns); f32 values live in pairs of bf16 slots.
OFF_DWDIAG = 0                    # [C, 9, C] diagonal depthwise weights
OFF_PWT = 9 * C                   # [C, O] pointwise weights, (c, o)
OFF_PWOC = OFF_PWT + O            # [C, 2, C] pointwise weights, (o%128, hf, c)
OFF_ID = OFF_PWOC + 2 * C         # [C, C] identity (PE transpose-by-matmul)
OFF_F32 = OFF_ID + C              # f32 section (even offset => 4B aligned)
OFF_DW9 = OFF_F32                 # 9 f32: raw dw weights for DVE taps
OFF_BN1 = OFF_DW9 + 18            # 2 f32: g1, b1
OFF_BN2 = OFF_BN1 + 4             # 4 f32: g2h0, g2h1, b2h0, b2h1
NCONST = OFF_BN2 + 8              # 1694 -> pad to even
NCONST += NCONST % 2


def _legalize_waits(nc):
    """Split multi-wait instructions: this walrus build's codegen accepts at
    most ONE sync wait per ISA instruction, while Tile's sem-assignment
    freely attaches several. Move all but one semaphore wait onto freshly
    inserted NoOps on the same engine directly before the instruction
    (waits are AND-semantics, so order is irrelevant)."""
    cnt = 0
    for bb in nc.main_func.blocks:
        new = []
        for ins in bb.instructions:
            si = ins.sync_info
            if si is not None and len(si.on_wait) > 1:
                sem_waits = [w for w in si.on_wait if w.sync_type == "semaphore"]
                other = [w for w in si.on_wait if w.sync_type != "semaphore"]
                keep = other + sem_waits[-1:] if not other else other
                move = sem_waits[:-1] if not other else sem_waits
                if len(keep) <= 1 and move:
                    for w in move:
                        cnt += 1
                        nop = mybir.InstNoOp(name=f"I-waitnop{cnt}", ins=[], outs=[])
                        nop.engine = ins.engine
                        nop.sync_info = mybir.SyncInfo(on_wait=[w], on_update=[])
                        new.append(nop)
                    ins.sync_info = mybir.SyncInfo(
                        on_wait=keep, on_update=list(si.on_update)
                    )
            new.append(ins)
        try:
            bb.instructions[:] = new
        except TypeError:
            bb.instructions = new
    return cnt


def _build_program(collectives=True, repeat=1, debug=False, legalize=True):
    # `collectives` kept for test-harness signature compat; this version uses
    # per-shard BN stats so the program has no cross-core communication.
    del collectives
    nc = bass.Bass(
        "TRN2",
        target_bir_lowering=False,
        debug=False,
        num_devices=1,
    )

    xp = nc.dram_tensor("xp", (IMG_PER_CORE, C, HP, WP), BF16, kind="ExternalInput").ap()
    cst = nc.dram_tensor("cst", (C, NCONST), BF16, kind="ExternalInput").ap()
    y = nc.dram_tensor(
        "y", (IMG_PER_CORE, O, H, W), BF16 if OUT_BF16 else F32,
        kind="ExternalOutput",
    ).ap()
    y_r = y.rearrange("n c h w -> n c (h w)")
    if debug:
        dbg_t = nc.dram_tensor("dbg_t", (C, PIX_TOTAL), BF16, kind="ExternalOutput").ap()
        dbg_s = nc.dram_tensor("dbg_s", (C, 96), F32, kind="ExternalOutput").ap()

    n_blocks = IMG_PER_CORE * (H // DMA_ROWS)   # 14

    with tile.TileContext(nc) as tc:
      for _rep in range(repeat):
        with (
            tc.tile_pool(name="consts", bufs=1) as consts,
            tc.tile_pool(name="big", bufs=1) as big,
            tc.tile_pool(name="xin", bufs=4) as xin,
            tc.tile_pool(name="stats", bufs=1) as stats,
            tc.tile_pool(name="yout", bufs=6) as yout,
            tc.tile_pool(name="ht", bufs=3) as big2,
            tc.tile_pool(name="psum", bufs=6, space="PSUM") as psum,
        ):
            # ---- constants: split DMA so the dw weights land first ---------
            cst_sb = consts.tile([C, NCONST], BF16)
            nc.scalar.dma_start(out=cst_sb[:, 0 : 9 * C], in_=cst[:, 0 : 9 * C])
            nc.scalar.dma_start(out=cst_sb[:, 9 * C :], in_=cst[:, 9 * C :])
            dwdiag_sb = cst_sb[:, OFF_DWDIAG : OFF_DWDIAG + 9 * C].rearrange(
                "p (t c) -> p t c", t=9
            )
            pwt_sb = cst_sb[:, OFF_PWT : OFF_PWT + O]
            pwoc_sb = cst_sb[:, OFF_PWOC : OFF_PWOC + 2 * C].rearrange(
                "p (h c) -> p h c", h=2
            )
            ident_sb = cst_sb[:, OFF_ID : OFF_ID + C]
            dw9_sb = cst_sb[:, OFF_DW9 : OFF_DW9 + 18].bitcast(F32)
            bn1gb_sb = cst_sb[:, OFF_BN1 : OFF_BN1 + 4].bitcast(F32)
            bn2gb_sb = cst_sb[:, OFF_BN2 : OFF_BN2 + 8].bitcast(F32)
            eps_sb = consts.tile([C, 1], F32)
            nc.vector.memset(eps_sb, EPS)

            # depthwise-conv output t (later h, in place), SBUF-resident
            t_sb = big.tile([C, PIX_TOTAL], BF16)
            t3 = t_sb.rearrange("p (r w) -> p r w", w=W)

            tsumW = stats.tile([C, 4 * n_blocks], F32)
            asq = stats.tile([C, n_blocks + 3], F32)
            sq_scr = stats.tile([C, DMA_ROWS * W], BF16)
            nc.vector.memset(tsumW, 0.0)
            nc.vector.memset(asq, 0.0)

            # ---- stage A: depthwise conv + BN1 partial stats --------------
            # GPSIMD cannot touch PSUM on HW, so evictions alternate per
            # block: even blocks fuse the eviction with a tap on DVE
            # (out = w*x + psum) and put the Square pass on ACT; odd blocks
            # evict on ACT (plain copy) and put the Square on DVE (TTR).
            # PE takes 5 taps, Pool 3 in-place SBUF taps (the last with the
            # sum(t) accumulator), DVE the remainder as TS(4x)+TT(2x) pairs.
            bi = 0
            for n in range(IMG_PER_CORE):
                for rblk in range(0, H, DMA_ROWS):
                    x_t = xin.tile([C, DMA_ROWS + 2, WP], BF16, tag="x")
                    if n == 0 and rblk == 0:
                        nc.sync.dma_start(
                            out=x_t[:, 0 : SUB_ROWS + 2, :],
                            in_=xp[n, :, 0 : SUB_ROWS + 2, :],
                        )
                        nc.sync.dma_start(
                            out=x_t[:, SUB_ROWS + 2 :, :],
                            in_=xp[n, :, SUB_ROWS + 2 : DMA_ROWS + 2, :],
                        )
                    else:
                        nc.sync.dma_start(
                            out=x_t, in_=xp[n, :, rblk : rblk + DMA_ROWS + 2, :]
                        )
                    last = bi == n_blocks - 1
                    # tap split (GPSIMD is ~10x slower than modeled on HW --
                    # unusable for bulk work). Every block: ACT pre-loads the
                    # PSUM bank with tap 0 (Copy with per-channel scale), PE
                    # accumulates taps 1..6 on top (start=False), DVE applies
                    # tap 7 fused with the eviction and tap 8 (STT, in place)
                    # carrying the sum(t) accumulator; ACT does the Square.
                    boff = n * PIX_PER_IMG + rblk * W
                    tblk = t_sb[:, boff : boff + DMA_ROWS * W]
                    tblk3 = tblk.rearrange("p (r w) -> p r w", r=DMA_ROWS)
                    # odd blocks add a DVE pair-tap for tap 6 (tensor_scalar
                    # 4x into tmp, tensor_add 2x in place after eviction)
                    heavy = bi % 2 == 1 and not last
                    n_pe = 6 if heavy else 7
                    tmp = None
                    if heavy:
                        di, dj = divmod(6, 3)
                        tmp = big2.tile([C, DMA_ROWS, W], BF16, tag="tmp", bufs=4)
                        nc.vector.tensor_scalar(
                            out=tmp,
                            in0=x_t[:, di : di + DMA_ROWS, dj : dj + W],
                            scalar1=dw9_sb[:, 6:7],
                            scalar2=None,
                            op0=mybir.AluOpType.mult,
                        )
                    # tap-major matmul order: same diag weight streams all
                    # four sub-blocks back-to-back (weight reuse on PE)
                    pts = []
                    for _k in range(DMA_ROWS // SUB_ROWS):
                        pt4 = psum.tile([C, PW_CHUNK], F32, tag="dw")
                        pts.append(pt4)
                    for t9 in range(n_pe):
                        di, dj = divmod(t9, 3)
                        for k2, sr in enumerate(range(0, DMA_ROWS, SUB_ROWS)):
                            rhs = x_t[:, sr + di : sr + di + SUB_ROWS, dj : dj + W]
                            nc.tensor.matmul(
                                pts[k2][:, : SUB_ROWS * W],
                                dwdiag_sb[:, t9, :],
                                rhs,
                                start=(t9 == 0),
                                stop=(t9 == n_pe - 1),
                            )
                    for sr in range(0, DMA_ROWS, SUB_ROWS):
                        k = sr // SUB_ROWS
                        off = boff + sr * W
                        ssl = t_sb[:, off : off + SUB_ROWS * W]
                        pt = pts[k]
                        # fused eviction + tap 7 on DVE
                        di, dj = divmod(7, 3)
                        nc.vector.scalar_tensor_tensor(
                            out=ssl,
                            in0=x_t[:, sr + di : sr + di + SUB_ROWS, dj : dj + W],
                            scalar=dw9_sb[:, 7:8],
                            in1=pt[:, : SUB_ROWS * W],
                            op0=mybir.AluOpType.mult,
                            op1=mybir.AluOpType.add,
                        )
                        if last:
                            # last block: per-448 sub-chains so the stats
                            # tail gating fold1 is short
                            di, dj = divmod(8, 3)
                            nc.vector.scalar_tensor_tensor(
                                out=ssl,
                                in0=x_t[
                                    :, sr + di : sr + di + SUB_ROWS, dj : dj + W
                                ],
                                scalar=dw9_sb[:, 8:9],
                                in1=ssl,
                                op0=mybir.AluOpType.mult,
                                op1=mybir.AluOpType.add,
                                accum_out=tsumW[:, 4 * bi + k : 4 * bi + k + 1],
                            )
                            nc.scalar.activation(
                                out=sq_scr[:, : SUB_ROWS * W],
                                in_=ssl,
                                func=mybir.ActivationFunctionType.Square,
                                accum_out=asq[:, bi + k : bi + k + 1],
                            )
                    if last:
                        bi += 1
                        continue
                    if tmp is not None:
                        nc.vector.tensor_add(out=tblk3, in0=tblk3, in1=tmp)
                    # final tap 8 on DVE (STT, in place) with sum(t) accum
                    di, dj = divmod(8, 3)
                    nc.vector.scalar_tensor_tensor(
                        out=tblk3,
                        in0=x_t[:, di : di + DMA_ROWS, dj : dj + W],
                        scalar=dw9_sb[:, 8:9],
                        in1=tblk3,
                        op0=mybir.AluOpType.mult,
                        op1=mybir.AluOpType.add,
                        accum_out=tsumW[:, 4 * bi : 4 * bi + 1],
                    )
                    # sum(t^2) for this block on ACT
                    nc.scalar.activation(
                        out=sq_scr,
                        in_=tblk,
                        func=mybir.ActivationFunctionType.Square,
                        accum_out=asq[:, bi : bi + 1],
                    )
                    bi += 1

            # ---- BN1 fold (per-shard stats) -------------------------------
            m1 = stats.tile([C, 1], F32)
            ex2 = stats.tile([C, 1], F32)
            var1 = stats.tile([C, 1], F32)
            a1 = stats.tile([C, 1], F32)
            c1 = stats.tile([C, 1], F32)
            nc.vector.reduce_sum(out=m1, in_=tsumW, axis=mybir.AxisListType.X)
            nc.vector.reduce_sum(out=ex2, in_=asq, axis=mybir.AxisListType.X)
            nc.vector.tensor_scalar_mul(out=m1, in0=m1, scalar1=1.0 / PIX_TOTAL)
            nc.vector.tensor_scalar_mul(out=ex2, in0=ex2, scalar1=1.0 / PIX_TOTAL)
            nc.vector.tensor_mul(out=var1, in0=m1, in1=m1)
            nc.vector.tensor_sub(out=var1, in0=ex2, in1=var1)
            nc.scalar.activation(
                out=var1, in_=var1,
                func=mybir.ActivationFunctionType.Sqrt,
                bias=eps_sb, scale=1.0,
            )
            nc.vector.reciprocal(out=a1, in_=var1)
            nc.vector.tensor_mul(out=a1, in0=a1, in1=bn1gb_sb[:, 0:1])
            nc.vector.tensor_mul(out=c1, in0=m1, in1=a1)
            nc.vector.tensor_sub(out=c1, in0=bn1gb_sb[:, 1:2], in1=c1)
            nc1 = stats.tile([C, 1], F32)
            nc.vector.tensor_scalar_mul(out=nc1, in0=c1, scalar1=-1.0)
            # keep the PE HAM window warm across the fold1 bubble: a tiny
            # matmul that depends on fold1 output (so it lands in the gap)
            if WARM:
                warm8 = stats.tile([C, 8], BF16)
                nc.vector.memset(warm8, 0.0)
                nc.vector.tensor_copy(out=warm8[:, 0:1], in_=c1)
                wp = psum.tile([C, 8], F32, tag="u", bufs=1)
                nc.tensor.matmul(wp, ident_sb, warm8, start=True, stop=True)

            if debug:
                nc.sync.dma_start(out=dbg_t, in_=t_sb)
            # ---- stage B: h = relu(a1*t + c1) in place; pw pass-1 stats ---
            # Per 2048-px chunk: h-apply (DVE 2x tensor_scalar pair, or a
            # single ACT Relu w/ accum on every 3rd chunk), then transpose
            # (XBAR DMA for even chunks, PE identity-matmul + Pool eviction
            # for odd), then Gram accumulation on PE.
            TR_CHUNK = 2048
            n_trc = (PIX_TOTAL + TR_CHUNK - 1) // TR_CHUNK
            hsum = stats.tile([C, n_trc], F32)
            gp = psum.tile([C, 128], F32, tag="g", bufs=1)
            bi_g = 0
            for ti in range(n_trc):
                hb = ti * TR_CHUNK
                sz = min(TR_CHUNK, PIX_TOTAL - hb)
                nblk_c = sz // 128
                hsl = t_sb[:, hb : hb + sz]
                if ti in (1, 5, 9):
                    # single-op h-apply on ACT (f32 internal, bf16 out)
                    nc.scalar.activation(
                        out=hsl, in_=hsl,
                        func=mybir.ActivationFunctionType.Relu,
                        bias=c1, scale=a1,
                        accum_out=hsum[:, ti : ti + 1],
                    )
                else:
                    # relu(a1*t + c1) = max(a1*t, -c1) + c1  (both 4x mode)
                    nc.vector.tensor_scalar(
                        out=hsl, in0=hsl, scalar1=a1, scalar2=nc1,
                        op0=mybir.AluOpType.mult, op1=mybir.AluOpType.max,
                    )
                    nc.vector.tensor_scalar(
                        out=hsl, in0=hsl, scalar1=c1, scalar2=0.0,
                        op0=mybir.AluOpType.add, op1=mybir.AluOpType.add,
                        accum_out=hsum[:, ti : ti + 1],
                    )
                if ti == 0:
                    # prefill stage-C tile (n=0, pblk=0, hf=0): matmuls only
                    # (evictions need fold2); fills the PE gap while the
                    # first transposes are in flight
                    prefill_ps = []
                    for sub in range(0, OUT_TILE, PW_CHUNK):
                        py = psum.tile([C, PW_CHUNK], F32, tag="dw")
                        nc.tensor.matmul(
                            py,
                            pwt_sb[:, 0:128],
                            t_sb[:, sub : sub + PW_CHUNK],
                            start=True, stop=True,
                        )
                        prefill_ps.append(py)
                htc = big2.tile([C, TR_CHUNK // 128, 128], BF16, tag="ht")
                if ti not in (2, 4, 6, 8):
                    nc.sync.dma_start_transpose(
                        out=htc[:, :nblk_c, :], in_=t_sb[:, hb : hb + sz]
                    )
                else:
                    for k4 in range(0, nblk_c, 4):
                        ptr = psum.tile([C, 512], F32, tag="dw")
                        for k in range(k4, k4 + 4):
                            nc.tensor.matmul(
                                ptr[:, (k - k4) * 128 : (k - k4 + 1) * 128],
                                t_sb[:, hb + 128 * k : hb + 128 * (k + 1)],
                                ident_sb,
                                start=True, stop=True,
                            )
                        nc.scalar.copy(out=htc[:, k4 : k4 + 4, :], in_=ptr)
                for k in range(nblk_c):
                    nc.tensor.matmul(
                        gp,
                        htc[:, k, :],
                        htc[:, k, :],
                        start=(bi_g == 0),
                        stop=(bi_g == NBLK - 1),
                    )
                    bi_g += 1

            # ---- BN2 fold: E[y] from sum(h); E[y^2] from the Gram ---------
            hs8 = stats.tile([C, 8], BF16)
            sh = stats.tile([C, 1], F32)
            nc.vector.memset(hs8, 0.0)
            nc.vector.reduce_sum(out=sh, in_=hsum, axis=mybir.AxisListType.X)
            nc.vector.tensor_copy(out=hs8[:, 0:1], in_=sh)
            g_sb = stats.tile([C, 128], BF16)
            z_scr = stats.tile([C, C], F32)
            sumsq = stats.tile([C, 2], F32)
            my = stats.tile([C, 2], F32)
            a2 = stats.tile([C, 2], F32)
            c2 = stats.tile([C, 2], F32)
            var2 = stats.tile([C, 1], F32)
            tmp2 = stats.tile([C, 1], F32)
            nc.vector.tensor_copy(out=g_sb, in_=gp)
            for hf in range(2):
                up = psum.tile([C, C], F32, tag="u", bufs=1)
                nc.tensor.matmul(
                    up, pwt_sb[:, hf * 128 : (hf + 1) * 128], g_sb,
                    start=True, stop=True,
                )
                # sum over c of U[o,c] * P[o,c]  ->  N * E[y^2]
                nc.vector.tensor_mul(out=z_scr, in0=up, in1=pwoc_sb[:, hf, :])
                nc.vector.reduce_sum(
                    out=sumsq[:, hf : hf + 1], in_=z_scr, axis=mybir.AxisListType.X
                )
                # N * E[y] at column 0 (rhs = [sum h | 0...] padded to 8)
                pm = psum.tile([C, 8], F32, tag="u", bufs=1)
                nc.tensor.matmul(
                    pm, pwt_sb[:, hf * 128 : (hf + 1) * 128], hs8,
                    start=True, stop=True,
                )
                nc.vector.tensor_scalar_mul(
                    out=my[:, hf : hf + 1], in0=pm[:, 0:1], scalar1=1.0 / PIX_TOTAL
                )
                nc.vector.tensor_scalar_mul(
                    out=var2, in0=sumsq[:, hf : hf + 1], scalar1=1.0 / PIX_TOTAL
                )
                nc.vector.tensor_mul(out=tmp2, in0=my[:, hf : hf + 1], in1=my[:, hf : hf + 1])
                nc.vector.tensor_sub(out=var2, in0=var2, in1=tmp2)
                nc.scalar.activation(
                    out=var2, in_=var2,
                    func=mybir.ActivationFunctionType.Sqrt,
                    bias=eps_sb, scale=1.0,
                )
                nc.vector.reciprocal(out=tmp2, in_=var2)
                nc.vector.tensor_mul(
                    out=a2[:, hf : hf + 1], in0=tmp2, in1=bn2gb_sb[:, hf : hf + 1]
                )
                nc.vector.tensor_mul(out=tmp2, in0=my[:, hf : hf + 1], in1=a2[:, hf : hf + 1])
                nc.vector.tensor_sub(
                    out=c2[:, hf : hf + 1],
                    in0=bn2gb_sb[:, 2 + hf : 3 + hf], in1=tmp2,
                )

            if debug:
                dbg_sb = stats.tile([C, 96], F32)
                nc.vector.memset(dbg_sb, 0.0)
                for dsrc, off in ((tsumW, 0), (asq, 56), (a1, 73), (c1, 74),
                                  (hsum, 75), (sumsq, 88), (my, 90),
                                  (a2, 92), (c2, 94)):
                    w_ = dsrc.shape[1]
                    nc.vector.tensor_copy(out=dbg_sb[:, off : off + w_], in_=dsrc)
                nc.sync.dma_start(out=dbg_s, in_=dbg_sb)
            nc2 = stats.tile([C, 2], F32)
            nc.vector.tensor_scalar_mul(out=nc2, in0=c2, scalar1=-1.0)

            # fold a2 into the pointwise weights: scale the (o,c)-layout
            # copy per partition=o, then XBAR-transpose back to (c,o) so the
            # stage-C eviction is relu(y' + c2) (one op on DVE).
            pws_oc = stats.tile([C, 2, C], BF16)
            for hf in range(2):
                nc.vector.tensor_scalar(
                    out=pws_oc[:, hf, :], in0=pwoc_sb[:, hf, :],
                    scalar1=a2[:, hf : hf + 1], scalar2=None,
                    op0=mybir.AluOpType.mult,
                )
            pws_T = stats.tile([C, 2, C], BF16)
            nc.sync.dma_start_transpose(
                out=pws_T, in_=pws_oc.rearrange("p a b -> p (a b)")
            )

            # ---- stage C: pointwise + fused BN2+ReLU eviction + store -----
            for n in range(IMG_PER_CORE):
                for pblk in range(0, PIX_PER_IMG, OUT_TILE):
                    bsz = min(OUT_TILE, PIX_PER_IMG - pblk)
                    for hf in range(2):
                        ot = yout.tile([C, OUT_TILE], BF16 if OUT_BF16 else F32, tag="yo")
                        for sub in range(0, bsz, PW_CHUNK):
                            sz = min(PW_CHUNK, bsz - sub)
                            off = n * PIX_PER_IMG + pblk + sub
                            ci_p = (n * PIX_PER_IMG + pblk + sub) // PW_CHUNK
                            # tile (n0, pblk0, hf0) was computed during stage
                            # B with the unscaled weights; (n0, pblk0, hf1)
                            # prefills during fold2 (also unscaled)
                            prefill = n == 0 and pblk == 0
                            if prefill and hf == 0:
                                py = prefill_ps[sub // PW_CHUNK]
                            else:
                                py = psum.tile([C, PW_CHUNK], F32, tag="dw")
                                nc.tensor.matmul(
                                    py[:, :sz],
                                    pwt_sb[:, hf * 128 : (hf + 1) * 128]
                                    if prefill
                                    else pws_T[:, hf, :],
                                    t_sb[:, off : off + sz],
                                    start=True, stop=True,
                                )
                            # eviction: relu(y' + c2), alternating ACT/DVE
                            ci_c = ci_p * 2 + hf
                            if prefill:
                                nc.scalar.activation(
                                    out=ot[:, sub : sub + sz], in_=py[:, :sz],
                                    func=mybir.ActivationFunctionType.Relu,
                                    bias=c2[:, hf : hf + 1],
                                    scale=a2[:, hf : hf + 1],
                                )
                            elif ci_c % 2 == 0:
                                nc.scalar.activation(
                                    out=ot[:, sub : sub + sz], in_=py[:, :sz],
                                    func=mybir.ActivationFunctionType.Relu,
                                    bias=c2[:, hf : hf + 1],
                                    scale=1.0,
                                )
                            else:
                                nc.vector.tensor_scalar(
                                    out=ot[:, sub : sub + sz], in0=py[:, :sz],
                                    scalar1=c2[:, hf : hf + 1],
                                    scalar2=0.0,
                                    op0=mybir.AluOpType.add,
                                    op1=mybir.AluOpType.max,
                                )
                        nc.sync.dma_start(
                            out=y_r[n, hf * 128 : (hf + 1) * 128, pblk : pblk + bsz],
                            in_=ot[:, :bsz],
                        )
    if legalize:
        _legalize_waits(nc)
    return nc


_NC_CACHE = []


def prepare(x, dw_w, dw_b, pw_b, pw_w, bn1_g, bn1_b, bn2_g, bn2_b, stride=1, **_):
    # dw_b / pw_b are absorbed by training-mode BN (they only shift the mean,
    # which BN subtracts) and are deliberately unused.
    x = np.asarray(x, dtype=np.float32)
    N = x.shape[0]
    assert x.shape == (16, C, H, W) and N == N_CORES * IMG_PER_CORE

    xp_full = np.zeros((N, C, HP, WP), dtype=ml_dtypes.bfloat16)
    xp_full[:, :, 1 : 1 + H, 1 : 1 + W] = x.astype(ml_dtypes.bfloat16)

    dw9 = np.asarray(dw_w, dtype=np.float32).reshape(C, 9)
    dwdiag = np.zeros((C, 9, C), dtype=ml_dtypes.bfloat16)
    idx = np.arange(C)
    for t in range(9):
        dwdiag[idx, t, idx] = dw9[:, t].astype(ml_dtypes.bfloat16)

    pw = np.asarray(pw_w, dtype=np.float32).reshape(O, C)
    g1 = np.asarray(bn1_g, np.float32)
    b1 = np.asarray(bn1_b, np.float32)
    g2 = np.asarray(bn2_g, np.float32)
    b2 = np.asarray(bn2_b, np.float32)

    cst = np.zeros((C, NCONST), dtype=ml_dtypes.bfloat16)
    cst[:, OFF_DWDIAG : OFF_DWDIAG + 9 * C] = dwdiag.reshape(C, 9 * C)
    cst[:, OFF_PWT : OFF_PWT + O] = pw.T.astype(ml_dtypes.bfloat16)
    # pw_oc[o%128, hf, c] = pw[hf*128+o, c]
    pwoc = np.stack([pw[:128, :], pw[128:, :]], axis=1)  # [128, 2, C]
    cst[:, OFF_PWOC : OFF_PWOC + 2 * C] = pwoc.reshape(128, 2 * C).astype(
        ml_dtypes.bfloat16
    )
    idc = np.arange(C)
    ident = np.zeros((C, C), dtype=ml_dtypes.bfloat16)
    ident[idc, idc] = 1.0
    cst[:, OFF_ID : OFF_ID + C] = ident
    # f32 sections: interleave dw9 at stride-2 slots so each f32 is aligned
    fview = cst[:, OFF_DW9 : OFF_DW9 + 18].view(np.float32)
    fview[:] = dw9
    fview = cst[:, OFF_BN1 : OFF_BN1 + 4].view(np.float32)
    fview[:, 0] = g1
    fview[:, 1] = b1
    fview = cst[:, OFF_BN2 : OFF_BN2 + 8].view(np.float32)
    fview[:, 0] = g2[:128]
    fview[:, 1] = g2[128:]
    fview[:, 2] = b2[:128]
    fview[:, 3] = b2[128:]

    if not _NC_CACHE:
        _NC_CACHE.append(_build_program())
    nc = _NC_CACHE[0]

    in_maps = []
    for k in range(N_CORES):
        in_maps.append(
            {
                "xp": np.ascontiguousarray(
                    xp_full[IMG_PER_CORE * k : IMG_PER_CORE * (k + 1)]
                ),
                "cst": cst,
            }
        )

    return nc, in_maps


def kernel(**inputs):
    nc, in_maps = prepare(**inputs)
    res = bass_utils.run_bass_kernel_spmd(
        nc, in_maps, core_ids=list(range(N_CORES))
    )
    out = np.concatenate(
        [np.asarray(r["y"], dtype=np.float32) for r in res.results], axis=0
    )
    return out



# revision 32
# speedup vs baseline: 1.3639x; 1.3639x over previous
"""Depthwise-separable conv block (dw3x3 + BN + ReLU + pw1x1 + BN + ReLU)
for Trainium2, data-parallel over batch across 8 NeuronCores with per-shard
BN statistics (explicitly sanctioned by the sharding hint). Measured rel
err ~1.5e-2 vs the sync-BN reference (gate: 2e-2), dominated by the
per-shard stats; bf16 contributes ~3e-3.

Design (v2, ~138us HW vs 284-339us for the sync-BN f32r baseline):
  - bf16 everywhere off-PSUM: x, t, h, weights, and the OUTPUT (host casts
    back to f32) -> input DMA halved, output DMA halved.
  - Stage A: depthwise conv = PSUM-accumulated diagonal matmuls (PE takes
    7 or 6 of the 9 taps per 16-row block, DVE the rest via
    scalar_tensor_tensor). sum(t) rides the accum_out of the last DVE tap;
    sum(t^2) is an ACT Square+accum pass. The last block is all-PE so
    fold1 is not gated on the serial DVE tail.
  - BN1 fold -> h = relu(a1*t+c1) in place on DVE (two 4x-mode
    tensor_scalar ops: max(a1*t,-c1)+c1), with sum(h) accumulated.
  - BN2 stats WITHOUT a second pointwise pass: G = H H^T via PE
    transpose-as-matmul (identity rhs, ~71ns per 128x128 block) + ACT
    evictions + 196 accumulating Gram matmuls. E[y] = P^T sum(h)/N
    (tiny matvec); E[y^2] = diag(P^T G P)/N via one matmul + multiply-
    reduce per half. A tiny warm-keeper matmul in fold1 keeps the PE HAM
    clock at 2.4 GHz across the stage boundary.
  - Stage C: single pointwise pass, BN2 affine+ReLU fused into the PSUM
    eviction (split ACT/DVE), bf16 output tiles, big output DMAs.
  - No collectives (per-shard stats) => no AllReduce latency.
"""

import numpy as np
import ml_dtypes

import concourse.bass as bass
import concourse.tile as tile
import concourse.mybir as mybir
from concourse import bass_utils

N_CORES = 8
# tuning flags (sim-ablatable)
TAPS_FRONT = False   # front-load DVE taps vs 7/6 alternation
LAST_PE = True       # last block all-PE taps
WARM = True          # fold1 PE warm-keeper matmul
B_SPLIT = False      # ht evictions split ACT/DVE (hurts: DVE is h-busy)
C_SPLIT = True       # stage C evictions split ACT/DVE
OUT_BF16 = True      # store output as bf16 (host casts to f32); halves out-DMA
CSPLIT_MOD = 2       # every Nth C-eviction goes to DVE
B_SPLIT_LATE = False # DVE takes ht evicts for late quads (hurt in sim)
C = 128          # input channels (= SBUF partitions)
O = 256          # output channels
H = W = 112
HP = WP = 114    # zero-padded input
IMG_PER_CORE = 2
PIX_PER_IMG = H * W                      # 12544
PIX_TOTAL = IMG_PER_CORE * PIX_PER_IMG   # 25088
NBLK = PIX_TOTAL // 128                  # 196 transpose blocks
EPS = 1e-5

F32 = mybir.dt.float32
BF16 = mybir.dt.bfloat16

DMA_ROWS = 16    # output rows per input DMA chunk (loads DMA_ROWS+2 rows)
SUB_ROWS = 4     # output rows per conv matmul chunk (N = 448)
PW_CHUNK = 512   # pixels per pointwise matmul (one PSUM bank of f32)
OUT_TILE = 2048  # pixels per output DMA tile
H_CHUNK = 2048   # pixels per h-apply / transpose chunk

# consts layout (bf16 columns); f32 values live in pairs of bf16 slots.
OFF_DWDIAG = 0                    # [C, 9, C] diagonal depthwise weights
OFF_PWT = 9 * C                   # [C, O] pointwise weights, (c, o)
OFF_PWOC = OFF_PWT + O            # [C, 2, C] pointwise weights, (o%128, hf, c)
OFF_ID = OFF_PWOC + 2 * C         # [C, C] identity (PE transpose-by-matmul)
OFF_F32 = OFF_ID + C              # f32 section (even offset => 4B aligned)
OFF_DW9 = OFF_F32                 # 9 f32: raw dw weights for DVE taps
OFF_BN1 = OFF_DW9 + 18            # 2 f32: g1, b1
OFF_BN2 = OFF_BN1 + 4             # 4 f32: g2h0, g2h1, b2h0, b2h1
NCONST = OFF_BN2 + 8              # 1694 -> pad to even
NCONST += NCONST % 2


def _legalize_waits(nc):
    """Split multi-wait instructions: this walrus build's codegen accepts at
    most ONE sync wait per ISA instruction, while Tile's sem-assignment
    freely attaches several. Move all but one semaphore wait onto freshly
    inserted NoOps on the same engine directly before the instruction
    (waits are AND-semantics, so order is irrelevant)."""
    cnt = 0
    for bb in nc.main_func.blocks:
        new = []
        for ins in bb.instructions:
            si = ins.sync_info
            if si is not None and len(si.on_wait) > 1:
                sem_waits = [w for w in si.on_wait if w.sync_type == "semaphore"]
                other = [w for w in si.on_wait if w.sync_type != "semaphore"]
                keep = other + sem_waits[-1:] if not other else other
                move = sem_waits[:-1] if not other else sem_waits
                if len(keep) <= 1 and move:
                    for w in move:
                        cnt += 1
                        nop = mybir.InstNoOp(name=f"I-waitnop{cnt}", ins=[], outs=[])
                        nop.engine = ins.engine
                        nop.sync_info = mybir.SyncInfo(on_wait=[w], on_update=[])
                        new.append(nop)
                    ins.sync_info = mybir.SyncInfo(
                        on_wait=keep, on_update=list(si.on_update)
                    )
            new.append(ins)
        try:
            bb.instructions[:] = new
        except TypeError:
            bb.instructions = new
    return cnt


def _build_program(collectives=True, repeat=1, debug=False, legalize=True):
    # `collectives` kept for test-harness signature compat; this version uses
    # per-shard BN stats so the program has no cross-core communication.
    del collectives
    nc = bass.Bass(
        "TRN2",
        target_bir_lowering=False,
        debug=False,
        num_devices=1,
    )

    xp = nc.dram_tensor("xp", (IMG_PER_CORE, C, HP, WP), BF16, kind="ExternalInput").ap()
    cst = nc.dram_tensor("cst", (C, NCONST), BF16, kind="ExternalInput").ap()
    y = nc.dram_tensor(
        "y", (IMG_PER_CORE, O, H, W), BF16 if OUT_BF16 else F32,
        kind="ExternalOutput",
    ).ap()
    y_r = y.rearrange("n c h w -> n c (h w)")
    if debug:
        dbg_t = nc.dram_tensor("dbg_t", (C, PIX_TOTAL), BF16, kind="ExternalOutput").ap()
        dbg_s = nc.dram_tensor("dbg_s", (C, 96), F32, kind="ExternalOutput").ap()

    n_blocks = IMG_PER_CORE * (H // DMA_ROWS)   # 14

    with tile.TileContext(nc) as tc:
      for _rep in range(repeat):
        with (
            tc.tile_pool(name="consts", bufs=1) as consts,
            tc.tile_pool(name="big", bufs=1) as big,
            tc.tile_pool(name="xin", bufs=6) as xin,
            tc.tile_pool(name="stats", bufs=1) as stats,
            tc.tile_pool(name="yout", bufs=8) as yout,
            tc.tile_pool(name="ht", bufs=4) as big2,
            tc.tile_pool(name="psum", bufs=6, space="PSUM") as psum,
        ):
            # ---- constants: split DMA so the dw weights land first ---------
            cst_sb = consts.tile([C, NCONST], BF16)
            nc.scalar.dma_start(out=cst_sb[:, 0 : 9 * C], in_=cst[:, 0 : 9 * C])
            nc.scalar.dma_start(out=cst_sb[:, 9 * C :], in_=cst[:, 9 * C :])
            dwdiag_sb = cst_sb[:, OFF_DWDIAG : OFF_DWDIAG + 9 * C].rearrange(
                "p (t c) -> p t c", t=9
            )
            pwt_sb = cst_sb[:, OFF_PWT : OFF_PWT + O]
            pwoc_sb = cst_sb[:, OFF_PWOC : OFF_PWOC + 2 * C].rearrange(
                "p (h c) -> p h c", h=2
            )
            ident_sb = cst_sb[:, OFF_ID : OFF_ID + C]
            dw9_sb = cst_sb[:, OFF_DW9 : OFF_DW9 + 18].bitcast(F32)
            bn1gb_sb = cst_sb[:, OFF_BN1 : OFF_BN1 + 4].bitcast(F32)
            bn2gb_sb = cst_sb[:, OFF_BN2 : OFF_BN2 + 8].bitcast(F32)
            eps_sb = consts.tile([C, 1], F32)
            nc.vector.memset(eps_sb, EPS)

            # depthwise-conv output t (later h, in place), SBUF-resident
            t_sb = big.tile([C, PIX_TOTAL], BF16)
            t3 = t_sb.rearrange("p (r w) -> p r w", w=W)

            tsumW = stats.tile([C, 4 * n_blocks], F32)
            asq = stats.tile([C, n_blocks + 3], F32)
            sq_scr = stats.tile([C, DMA_ROWS * W], BF16)
            nc.vector.memset(tsumW, 0.0)
            nc.vector.memset(asq, 0.0)

            # ---- stage A: depthwise conv + BN1 partial stats --------------
            # GPSIMD cannot touch PSUM on HW, so evictions alternate per
            # block: even blocks fuse the eviction with a tap on DVE
            # (out = w*x + psum) and put the Square pass on ACT; odd blocks
            # evict on ACT (plain copy) and put the Square on DVE (TTR).
            # PE takes 5 taps, Pool 3 in-place SBUF taps (the last with the
            # sum(t) accumulator), DVE the remainder as TS(4x)+TT(2x) pairs.
            bi = 0
            for n in range(IMG_PER_CORE):
                for rblk in range(0, H, DMA_ROWS):
                    x_t = xin.tile([C, DMA_ROWS + 2, WP], BF16, tag="x")
                    if n == 0 and rblk == 0:
                        nc.sync.dma_start(
                            out=x_t[:, 0 : SUB_ROWS + 2, :],
                            in_=xp[n, :, 0 : SUB_ROWS + 2, :],
                        )
                        nc.sync.dma_start(
                            out=x_t[:, SUB_ROWS + 2 :, :],
                            in_=xp[n, :, SUB_ROWS + 2 : DMA_ROWS + 2, :],
                        )
                    else:
                        nc.sync.dma_start(
                            out=x_t, in_=xp[n, :, rblk : rblk + DMA_ROWS + 2, :]
                        )
                    last = bi == n_blocks - 1
                    # tap split (GPSIMD is ~10x slower than modeled on HW --
                    # unusable for bulk work). Every block: ACT pre-loads the
                    # PSUM bank with tap 0 (Copy with per-channel scale), PE
                    # accumulates taps 1..6 on top (start=False), DVE applies
                    # tap 7 fused with the eviction and tap 8 (STT, in place)
                    # carrying the sum(t) accumulator; ACT does the Square.
                    boff = n * PIX_PER_IMG + rblk * W
                    tblk = t_sb[:, boff : boff + DMA_ROWS * W]
                    tblk3 = tblk.rearrange("p (r w) -> p r w", r=DMA_ROWS)
                    # odd blocks add a DVE pair-tap for tap 6 (tensor_scalar
                    # 4x into tmp, tensor_add 2x in place after eviction)
                    heavy = bi % 2 == 1 and not last
                    n_pe = 6 if heavy else 7
                    tmp = None
                    if heavy:
                        di, dj = divmod(6, 3)
                        tmp = big2.tile([C, DMA_ROWS, W], BF16, tag="tmp", bufs=4)
                        nc.vector.tensor_scalar(
                            out=tmp,
                            in0=x_t[:, di : di + DMA_ROWS, dj : dj + W],
                            scalar1=dw9_sb[:, 6:7],
                            scalar2=None,
                            op0=mybir.AluOpType.mult,
                        )
                    for sr in range(0, DMA_ROWS, SUB_ROWS):
                        k = sr // SUB_ROWS
                        off = boff + sr * W
                        ssl = t_sb[:, off : off + SUB_ROWS * W]
                        pt = psum.tile([C, PW_CHUNK], F32, tag="dw")
                        for t9 in range(n_pe):
                            di, dj = divmod(t9, 3)
                            rhs = x_t[:, sr + di : sr + di + SUB_ROWS, dj : dj + W]
                            nc.tensor.matmul(
                                pt[:, : SUB_ROWS * W],
                                dwdiag_sb[:, t9, :],
                                rhs,
                                start=(t9 == 0),
                                stop=(t9 == n_pe - 1),
                            )
                        # fused eviction + tap 7 on DVE
                        di, dj = divmod(7, 3)
                        nc.vector.scalar_tensor_tensor(
                            out=ssl,
                            in0=x_t[:, sr + di : sr + di + SUB_ROWS, dj : dj + W],
                            scalar=dw9_sb[:, 7:8],
                            in1=pt[:, : SUB_ROWS * W],
                            op0=mybir.AluOpType.mult,
                            op1=mybir.AluOpType.add,
                        )
                        if last:
                            # last block: per-448 sub-chains so the stats
                            # tail gating fold1 is short
                            di, dj = divmod(8, 3)
                            nc.vector.scalar_tensor_tensor(
                                out=ssl,
                                in0=x_t[
                                    :, sr + di : sr + di + SUB_ROWS, dj : dj + W
                                ],
                                scalar=dw9_sb[:, 8:9],
                                in1=ssl,
                                op0=mybir.AluOpType.mult,
                                op1=mybir.AluOpType.add,
                                accum_out=tsumW[:, 4 * bi + k : 4 * bi + k + 1],
                            )
                            nc.scalar.activation(
                                out=sq_scr[:, : SUB_ROWS * W],
                                in_=ssl,
                                func=mybir.ActivationFunctionType.Square,
                                accum_out=asq[:, bi + k : bi + k + 1],
                            )
                    if last:
                        bi += 1
                        continue
                    if tmp is not None:
                        nc.vector.tensor_add(out=tblk3, in0=tblk3, in1=tmp)
                    # final tap 8 on DVE (STT, in place) with sum(t) accum
                    di, dj = divmod(8, 3)
                    nc.vector.scalar_tensor_tensor(
                        out=tblk3,
                        in0=x_t[:, di : di + DMA_ROWS, dj : dj + W],
                        scalar=dw9_sb[:, 8:9],
                        in1=tblk3,
                        op0=mybir.AluOpType.mult,
                        op1=mybir.AluOpType.add,
                        accum_out=tsumW[:, 4 * bi : 4 * bi + 1],
                    )
                    # sum(t^2) for this block on ACT
                    nc.scalar.activation(
                        out=sq_scr,
                        in_=tblk,
                        func=mybir.ActivationFunctionType.Square,
                        accum_out=asq[:, bi : bi + 1],
                    )
                    bi += 1

            # ---- BN1 fold (per-shard stats) -------------------------------
            m1 = stats.tile([C, 1], F32)
            ex2 = stats.tile([C, 1], F32)
            var1 = stats.tile([C, 1], F32)
            a1 = stats.tile([C, 1], F32)
            c1 = stats.tile([C, 1], F32)
            nc.vector.reduce_sum(out=m1, in_=tsumW, axis=mybir.AxisListType.X)
            nc.vector.reduce_sum(out=ex2, in_=asq, axis=mybir.AxisListType.X)
            nc.vector.tensor_scalar_mul(out=m1, in0=m1, scalar1=1.0 / PIX_TOTAL)
            nc.vector.tensor_scalar_mul(out=ex2, in0=ex2, scalar1=1.0 / PIX_TOTAL)
            nc.vector.tensor_mul(out=var1, in0=m1, in1=m1)
            nc.vector.tensor_sub(out=var1, in0=ex2, in1=var1)
            nc.scalar.activation(
                out=var1, in_=var1,
                func=mybir.ActivationFunctionType.Sqrt,
                bias=eps_sb, scale=1.0,
            )
            nc.vector.reciprocal(out=a1, in_=var1)
            nc.vector.tensor_mul(out=a1, in0=a1, in1=bn1gb_sb[:, 0:1])
            nc.vector.tensor_mul(out=c1, in0=m1, in1=a1)
            nc.vector.tensor_sub(out=c1, in0=bn1gb_sb[:, 1:2], in1=c1)
            nc1 = stats.tile([C, 1], F32)
            nc.vector.tensor_scalar_mul(out=nc1, in0=c1, scalar1=-1.0)
            # keep the PE HAM window warm across the fold1 bubble: a tiny
            # matmul that depends on fold1 output (so it lands in the gap)
            if WARM:
                warm8 = stats.tile([C, 8], BF16)
                nc.vector.memset(warm8, 0.0)
                nc.vector.tensor_copy(out=warm8[:, 0:1], in_=c1)
                wp = psum.tile([C, 8], F32, tag="u", bufs=1)
                nc.tensor.matmul(wp, ident_sb, warm8, start=True, stop=True)

            if debug:
                nc.sync.dma_start(out=dbg_t, in_=t_sb)
            # ---- stage B: h = relu(a1*t + c1) in place; pw pass-1 stats ---
            # Per 2048-px chunk: h-apply (DVE 2x tensor_scalar pair, or a
            # single ACT Relu w/ accum on every 3rd chunk), then transpose
            # (XBAR DMA for even chunks, PE identity-matmul + Pool eviction
            # for odd), then Gram accumulation on PE.
            TR_CHUNK = 2048
            n_trc = (PIX_TOTAL + TR_CHUNK - 1) // TR_CHUNK
            hsum = stats.tile([C, n_trc], F32)
            gp = psum.tile([C, 128], F32, tag="g", bufs=1)
            bi_g = 0
            for ti in range(n_trc):
                hb = ti * TR_CHUNK
                sz = min(TR_CHUNK, PIX_TOTAL - hb)
                nblk_c = sz // 128
                hsl = t_sb[:, hb : hb + sz]
                if ti in (1, 5, 9):
                    # single-op h-apply on ACT (f32 internal, bf16 out)
                    nc.scalar.activation(
                        out=hsl, in_=hsl,
                        func=mybir.ActivationFunctionType.Relu,
                        bias=c1, scale=a1,
                        accum_out=hsum[:, ti : ti + 1],
                    )
                else:
                    # relu(a1*t + c1) = max(a1*t, -c1) + c1  (both 4x mode)
                    nc.vector.tensor_scalar(
                        out=hsl, in0=hsl, scalar1=a1, scalar2=nc1,
                        op0=mybir.AluOpType.mult, op1=mybir.AluOpType.max,
                    )
                    nc.vector.tensor_scalar(
                        out=hsl, in0=hsl, scalar1=c1, scalar2=0.0,
                        op0=mybir.AluOpType.add, op1=mybir.AluOpType.add,
                        accum_out=hsum[:, ti : ti + 1],
                    )
                if ti == 0:
                    # prefill stage-C tile (n=0, pblk=0, hf=0): matmuls only
                    # (evictions need fold2); fills the PE gap while the
                    # first transposes are in flight
                    prefill_ps = []
                    for sub in range(0, OUT_TILE, PW_CHUNK):
                        py = psum.tile([C, PW_CHUNK], F32, tag="dw")
                        nc.tensor.matmul(
                            py,
                            pwt_sb[:, 0:128],
                            t_sb[:, sub : sub + PW_CHUNK],
                            start=True, stop=True,
                        )
                        prefill_ps.append(py)
                htc = big2.tile([C, TR_CHUNK // 128, 128], BF16, tag="ht")
                if ti not in (2, 4, 6, 8):
                    nc.sync.dma_start_transpose(
                        out=htc[:, :nblk_c, :], in_=t_sb[:, hb : hb + sz]
                    )
                else:
                    for k4 in range(0, nblk_c, 4):
                        ptr = psum.tile([C, 512], F32, tag="dw")
                        for k in range(k4, k4 + 4):
                            nc.tensor.matmul(
                                ptr[:, (k - k4) * 128 : (k - k4 + 1) * 128],
                                t_sb[:, hb + 128 * k : hb + 128 * (k + 1)],
                                ident_sb,
                                start=True, stop=True,
                            )
                        nc.scalar.copy(out=htc[:, k4 : k4 + 4, :], in_=ptr)
                for k in range(nblk_c):
                    nc.tensor.matmul(
                        gp,
                        htc[:, k, :],
                        htc[:, k, :],
                        start=(bi_g == 0),
                        stop=(bi_g == NBLK - 1),
                    )
                    bi_g += 1

            # ---- BN2 fold: E[y] from sum(h); E[y^2] from the Gram ---------
            hs8 = stats.tile([C, 8], BF16)
            sh = stats.tile([C, 1], F32)
            nc.vector.memset(hs8, 0.0)
            nc.vector.reduce_sum(out=sh, in_=hsum, axis=mybir.AxisListType.X)
            nc.vector.tensor_copy(out=hs8[:, 0:1], in_=sh)
            g_sb = stats.tile([C, 128], BF16)
            z_scr = stats.tile([C, C], F32)
            sumsq = stats.tile([C, 2], F32)
            my = stats.tile([C, 2], F32)
            a2 = stats.tile([C, 2], F32)
            c2 = stats.tile([C, 2], F32)
            var2 = stats.tile([C, 1], F32)
            tmp2 = stats.tile([C, 1], F32)
            nc.vector.tensor_copy(out=g_sb, in_=gp)
            for hf in range(2):
                up = psum.tile([C, C], F32, tag="u", bufs=1)
                nc.tensor.matmul(
                    up, pwt_sb[:, hf * 128 : (hf + 1) * 128], g_sb,
                    start=True, stop=True,
                )
                # sum over c of U[o,c] * P[o,c]  ->  N * E[y^2]
                nc.vector.tensor_mul(out=z_scr, in0=up, in1=pwoc_sb[:, hf, :])
                nc.vector.reduce_sum(
                    out=sumsq[:, hf : hf + 1], in_=z_scr, axis=mybir.AxisListType.X
                )
                # N * E[y] at column 0 (rhs = [sum h | 0...] padded to 8)
                pm = psum.tile([C, 8], F32, tag="u", bufs=1)
                nc.tensor.matmul(
                    pm, pwt_sb[:, hf * 128 : (hf + 1) * 128], hs8,
                    start=True, stop=True,
                )
                nc.vector.tensor_scalar_mul(
                    out=my[:, hf : hf + 1], in0=pm[:, 0:1], scalar1=1.0 / PIX_TOTAL
                )
                nc.vector.tensor_scalar_mul(
                    out=var2, in0=sumsq[:, hf : hf + 1], scalar1=1.0 / PIX_TOTAL
                )
                nc.vector.tensor_mul(out=tmp2, in0=my[:, hf : hf + 1], in1=my[:, hf : hf + 1])
                nc.vector.tensor_sub(out=var2, in0=var2, in1=tmp2)
                nc.scalar.activation(
                    out=var2, in_=var2,
                    func=mybir.ActivationFunctionType.Sqrt,
                    bias=eps_sb, scale=1.0,
                )
                nc.vector.reciprocal(out=tmp2, in_=var2)
                nc.vector.tensor_mul(
                    out=a2[:, hf : hf + 1], in0=tmp2, in1=bn2gb_sb[:, hf : hf + 1]
                )
                nc.vector.tensor_mul(out=tmp2, in0=my[:, hf : hf + 1], in1=a2[:, hf : hf + 1])
                nc.vector.tensor_sub(
                    out=c2[:, hf : hf + 1],
                    in0=bn2gb_sb[:, 2 + hf : 3 + hf], in1=tmp2,
                )

            if debug:
                dbg_sb = stats.tile([C, 96], F32)
                nc.vector.memset(dbg_sb, 0.0)
                for dsrc, off in ((tsumW, 0), (asq, 56), (a1, 73), (c1, 74),
                                  (hsum, 75), (sumsq, 88), (my, 90),
                                  (a2, 92), (c2, 94)):
                    w_ = dsrc.shape[1]
                    nc.vector.tensor_copy(out=dbg_sb[:, off : off + w_], in_=dsrc)
                nc.sync.dma_start(out=dbg_s, in_=dbg_sb)
            nc2 = stats.tile([C, 2], F32)
            nc.vector.tensor_scalar_mul(out=nc2, in0=c2, scalar1=-1.0)

            # fold a2 into the pointwise weights: scale the (o,c)-layout
            # copy per partition=o, then XBAR-transpose back to (c,o) so the
            # stage-C eviction is relu(y' + c2) (one op on DVE).
            pws_oc = stats.tile([C, 2, C], BF16)
            for hf in range(2):
                nc.vector.tensor_scalar(
                    out=pws_oc[:, hf, :], in0=pwoc_sb[:, hf, :],
                    scalar1=a2[:, hf : hf + 1], scalar2=None,
                    op0=mybir.AluOpType.mult,
                )
            pws_T = stats.tile([C, 2, C], BF16)
            nc.sync.dma_start_transpose(
                out=pws_T, in_=pws_oc.rearrange("p a b -> p (a b)")
            )

            # ---- stage C: pointwise + fused BN2+ReLU eviction + store -----
            for n in range(IMG_PER_CORE):
                for pblk in range(0, PIX_PER_IMG, OUT_TILE):
                    bsz = min(OUT_TILE, PIX_PER_IMG - pblk)
                    for hf in range(2):
                        ot = yout.tile([C, OUT_TILE], BF16 if OUT_BF16 else F32, tag="yo")
                        for sub in range(0, bsz, PW_CHUNK):
                            sz = min(PW_CHUNK, bsz - sub)
                            off = n * PIX_PER_IMG + pblk + sub
                            ci_p = (n * PIX_PER_IMG + pblk + sub) // PW_CHUNK
                            # tile (n0, pblk0, hf0) was computed during stage
                            # B with the unscaled weights; (n0, pblk0, hf1)
                            # prefills during fold2 (also unscaled)
                            prefill = n == 0 and pblk == 0
                            if prefill and hf == 0:
                                py = prefill_ps[sub // PW_CHUNK]
                            else:
                                py = psum.tile([C, PW_CHUNK], F32, tag="dw")
                                nc.tensor.matmul(
                                    py[:, :sz],
                                    pwt_sb[:, hf * 128 : (hf + 1) * 128]
                                    if prefill
                                    else pws_T[:, hf, :],
                                    t_sb[:, off : off + sz],
                                    start=True, stop=True,
                                )
                            # eviction: relu(y' + c2), alternating ACT/DVE
                            ci_c = ci_p * 2 + hf
                            if prefill:
                                nc.scalar.activation(
                                    out=ot[:, sub : sub + sz], in_=py[:, :sz],
                                    func=mybir.ActivationFunctionType.Relu,
                                    bias=c2[:, hf : hf + 1],
                                    scale=a2[:, hf : hf + 1],
                                )
                            elif ci_c % 2 == 0:
                                nc.scalar.activation(
                                    out=ot[:, sub : sub + sz], in_=py[:, :sz],
                                    func=mybir.ActivationFunctionType.Relu,
                                    bias=c2[:, hf : hf + 1],
                                    scale=1.0,
                                )
                            else:
                                nc.vector.tensor_scalar(
                                    out=ot[:, sub : sub + sz], in0=py[:, :sz],
                                    scalar1=c2[:, hf : hf + 1],
                                    scalar2=0.0,
                                    op0=mybir.AluOpType.add,
                                    op1=mybir.AluOpType.max,
                                )
                        nc.sync.dma_start(
                            out=y_r[n, hf * 128 : (hf + 1) * 128, pblk : pblk + bsz],
                            in_=ot[:, :bsz],
                        )
    if legalize:
        _legalize_waits(nc)
    return nc


_NC_CACHE = []


def prepare(x, dw_w, dw_b, pw_b, pw_w, bn1_g, bn1_b, bn2_g, bn2_b, stride=1, **_):
    # dw_b / pw_b are absorbed by training-mode BN (they only shift the mean,
    # which BN subtracts) and are deliberately unused.
    x = np.asarray(x, dtype=np.float32)
    N = x.shape[0]
    assert x.shape == (16, C, H, W) and N == N_CORES * IMG_PER_CORE

    xp_full = np.zeros((N, C, HP, WP), dtype=ml_dtypes.bfloat16)
    xp_full[:, :, 1 : 1 + H, 1 : 1 + W] = x.astype(ml_dtypes.bfloat16)

    dw9 = np.asarray(dw_w, dtype=np.float32).reshape(C, 9)
    dwdiag = np.zeros((C, 9, C), dtype=ml_dtypes.bfloat16)
    idx = np.arange(C)
    for t in range(9):
        dwdiag[idx, t, idx] = dw9[:, t].astype(ml_dtypes.bfloat16)

    pw = np.asarray(pw_w, dtype=np.float32).reshape(O, C)
    g1 = np.asarray(bn1_g, np.float32)
    b1 = np.asarray(bn1_b, np.float32)
    g2 = np.asarray(bn2_g, np.float32)
    b2 = np.asarray(bn2_b, np.float32)

    cst = np.zeros((C, NCONST), dtype=ml_dtypes.bfloat16)
    cst[:, OFF_DWDIAG : OFF_DWDIAG + 9 * C] = dwdiag.reshape(C, 9 * C)
    cst[:, OFF_PWT : OFF_PWT + O] = pw.T.astype(ml_dtypes.bfloat16)
    # pw_oc[o%128, hf, c] = pw[hf*128+o, c]
    pwoc = np.stack([pw[:128, :], pw[128:, :]], axis=1)  # [128, 2, C]
    cst[:, OFF_PWOC : OFF_PWOC + 2 * C] = pwoc.reshape(128, 2 * C).astype(
        ml_dtypes.bfloat16
    )
    idc = np.arange(C)
    ident = np.zeros((C, C), dtype=ml_dtypes.bfloat16)
    ident[idc, idc] = 1.0
    cst[:, OFF_ID : OFF_ID + C] = ident
    # f32 sections: interleave dw9 at stride-2 slots so each f32 is aligned
    fview = cst[:, OFF_DW9 : OFF_DW9 + 18].view(np.float32)
    fview[:] = dw9
    fview = cst[:, OFF_BN1 : OFF_BN1 + 4].view(np.float32)
    fview[:, 0] = g1
    fview[:, 1] = b1
    fview = cst[:, OFF_BN2 : OFF_BN2 + 8].view(np.float32)
    fview[:, 0] = g2[:128]
    fview[:, 1] = g2[128:]
    fview[:, 2] = b2[:128]
    fview[:, 3] = b2[128:]

    if not _NC_CACHE:
        _NC_CACHE.append(_build_program())
    nc = _NC_CACHE[0]

    in_maps = []
    for k in range(N_CORES):
        in_maps.append(
            {
                "xp": np.ascontiguousarray(
                    xp_full[IMG_PER_CORE * k : IMG_PER_CORE * (k + 1)]
                ),
                "cst": cst,
            }
        )

    return nc, in_maps


def kernel(**inputs):
    nc, in_maps = prepare(**inputs)
    res = bass_utils.run_bass_kernel_spmd(
        nc, in_maps, core_ids=list(range(N_CORES))
    )
    out = np.concatenate(
        [np.asarray(r["y"], dtype=np.float32) for r in res.results], axis=0
    )
    return out

